# revision 20
# baseline (speedup 1.0000x reference)
"""PolynormerAttention Trainium2 kernel (8 NeuronCores).

Contract: kernel(**inputs) takes the FULL tensors (x [4,16384,512] f32,
mask [4,16384] bool, weights) and returns the FULL output [4,16384,512] f32.

Sharding: core c handles batch c//2, row-half c%2 (8192 rows). The per-head
reductions kv [64,64,H] / k_sum [64,H] are completed with an in-kernel
pairwise AllReduce (replica groups [[0,1],[2,3],[4,5],[6,7]]).

Host-side prep (free — only HW kernel time matters):
  - head-major channel permutation (flat' = h*64+d) applied to Wk/Wv/Wh
    columns, Wo rows, bh/ln_g/ln_b, so each head is 64 contiguous columns.
  - x is pre-transposed per core (xT [512, 8192]) so the kernel never
    transposes x on-chip (contraction over CH needs CH on partitions).
  - matmul operands are fp16 (fp32 PSUM accumulation; values are all far
    inside fp16 range); elementwise math stays fp32.

Device program (identical on all cores):
  pass 1: k = sigmoid(x@Wk), v = (x@Wv)*mask as [n, dh] tiles;
          kv/ksum accumulated in PSUM over all rows via per-head-pair
          matmuls with rhs [v_pair | ones]; pairwise AllReduce.
  pass 2: kT = sigmoid((x@Wk).T) computed directly dh-major (lhsT = Wk);
          h = x@Wh + (bh+beta); num/den in one matmul per head-pair with a
          block-diagonal [kv|ksum] rhs; attn = num * recip(den); layernorm
          (Sqrt batched per 512-row block to avoid ACT table thrash);
          * h; PE-transpose; @ Wo (+bo folded in as a K=1 matmul); relu.
"""

import numpy as np

HEADS = 8
HEAD_CH = 64
CH = 512
INNER = 512
BETA = 0.9
B = 4
N = 16384
N_CORES = 8
LN_EPS = 1e-5


def _perm():
    # new flat index j = h*64 + d  <-  old flat index d*8 + h
    j = np.arange(INNER)
    h = j // HEAD_CH
    d = j % HEAD_CH
    return d * HEADS + h


def _strided_cols(t, start, step, count):
    """AP selecting columns start, start+step, ... of a 2D/flattened tile."""
    import concourse.bass as bass
    a = t[:, 0:1] if not isinstance(t, bass.AP) else t
    p_ent = a.ap[0]
    f_stride = a.ap[-1][0]
    return bass.AP(tensor=a.tensor, offset=a.offset + start * f_stride,
                   ap=[p_ent, [step * f_stride, count]])


def build_program(rows, apply_ln_affine, debug_taps=False):
    """Build the per-core Bass program for `rows` rows/core."""
    import concourse.bass as bass
    import concourse.tile as tile
    from concourse import mybir, bacc
    from concourse.masks import make_identity

    F32 = mybir.dt.float32
    F16 = mybir.dt.float16
    AF = mybir.ActivationFunctionType

    assert rows % 512 == 0
    n_blocks = rows // 512

    nc = bacc.Bacc("TRN2", target_bir_lowering=False, debug=False,
                   num_devices=N_CORES)

    xT = nc.dram_tensor("xT", [CH, rows], F16, kind="ExternalInput")
    maskf = nc.dram_tensor("maskf", [128, rows // 128], F32, kind="ExternalInput")
    wk_d = nc.dram_tensor("wk", [CH, INNER], F16, kind="ExternalInput")
    wv_d = nc.dram_tensor("wv", [CH, INNER], F16, kind="ExternalInput")
    wh_d = nc.dram_tensor("wh", [CH, INNER], F16, kind="ExternalInput")
    wo_d = nc.dram_tensor("wo", [INNER, INNER], F16, kind="ExternalInput")
    bhb_d = nc.dram_tensor("bhb", [1, INNER], F32, kind="ExternalInput")
    bo_d = nc.dram_tensor("bo_r", [1, INNER], F16, kind="ExternalInput")
    if apply_ln_affine:
        lng_d = nc.dram_tensor("lng", [1, INNER], F32, kind="ExternalInput")
        lnb_d = nc.dram_tensor("lnb", [1, INNER], F32, kind="ExternalInput")
    out_d = nc.dram_tensor("out", [rows, INNER], F32, kind="ExternalOutput")
    if debug_taps:
        dbg_cc = nc.dram_tensor("dbg_cc", [128, 516], F32, kind="ExternalOutput")
        dbg_attn = nc.dram_tensor("dbg_attn", [128, INNER], F32,
                                  kind="ExternalOutput")
        dbg_h = nc.dram_tensor("dbg_h", [128, INNER], F32, kind="ExternalOutput")
        dbg_k = nc.dram_tensor("dbg_k", [128, INNER], F32, kind="ExternalOutput")
        dbg_kt = nc.dram_tensor("dbg_kt", [128, INNER], F32, kind="ExternalOutput")
        dbg_pre = nc.dram_tensor("dbg_pre", [128, INNER], F32,
                                 kind="ExternalOutput")

    def bcast128(ap_1xN):
        return bass.AP(tensor=ap_1xN.tensor, offset=ap_1xN.offset,
                       ap=[[0, 128]] + list(ap_1xN.ap[1:]))

    with tile.TileContext(nc) as tc:
        with (
            tc.tile_pool(name="singles", bufs=1) as singles,
            tc.tile_pool(name="dram", bufs=1, space="DRAM") as dram,
        ):
            # ---- weights / constants ----
            wk = singles.tile([128, 4, INNER], F16, tag="wk")
            wv = singles.tile([128, 4, INNER], F16, tag="wv")
            wh = singles.tile([128, 4, INNER], F16, tag="wh")
            wo = singles.tile([128, 4, INNER], F16, tag="wo")
            for w_sb, w_dram in ((wk, wk_d), (wv, wv_d), (wh, wh_d), (wo, wo_d)):
                nc.sync.dma_start(
                    out=w_sb,
                    in_=w_dram[:, :].rearrange("(c p) i -> p c i", p=128))
            mask_sb = singles.tile([128, rows // 128], F32, tag="mask")
            nc.sync.dma_start(out=mask_sb, in_=maskf[:, :])
            bo_sb = singles.tile([1, INNER], F16, tag="bo")
            nc.sync.dma_start(out=bo_sb, in_=bo_d[:, :])
            bhb_r = singles.tile([128, INNER], F32, tag="bhbr")
            nc.gpsimd.dma_start(out=bhb_r, in_=bcast128(bhb_d[:, :]))
            if apply_ln_affine:
                lng_r = singles.tile([128, INNER], F32, tag="lngr")
                lnb_r = singles.tile([128, INNER], F32, tag="lnbr")
                nc.gpsimd.dma_start(out=lng_r, in_=bcast128(lng_d[:, :]))
                nc.gpsimd.dma_start(out=lnb_r, in_=bcast128(lnb_d[:, :]))
            ident = singles.tile([128, 128], F16, tag="ident")
            make_identity(nc, ident)
            ones_lhsT = singles.tile([1, 128], F16, tag="ones1")
            nc.gpsimd.memset(ones_lhsT, 1.0)
            eps_t = singles.tile([128, 1], F32, tag="eps")
            nc.vector.memset(eps_t, LN_EPS)

            # ============================ PASS 1 ============================
            with (
                tc.tile_pool(name="p1xt", bufs=2) as p1xt,
                tc.tile_pool(name="p1k", bufs=3) as p1k,
                tc.tile_pool(name="p1v", bufs=3) as p1v,
                tc.tile_pool(name="ps1", bufs=2, space="PSUM") as ps1,
                tc.tile_pool(name="pskv", bufs=1, space="PSUM") as pskv,
            ):
                kvacc = [pskv.tile([128, 129], F32, tag=f"kvacc{q}",
                                   name=f"kvacc{q}")
                         for q in range(4)]
                n_tiles = 4 * n_blocks
                for blk in range(n_blocks):
                    xt = []
                    for c in range(4):
                        t = p1xt.tile([128, 512], F16, tag=f"xt{c}")
                        nc.sync.dma_start(
                            out=t, in_=xT[c * 128:(c + 1) * 128,
                                          blk * 512:(blk + 1) * 512])
                        xt.append(t)
                    for tt in range(4):
                        gt = blk * 4 + tt
                        ps_k = ps1.tile([128, 512], F32, tag="psk")
                        ps_v = ps1.tile([128, 512], F32, tag="psv")
                        for c in range(4):
                            lhsT = xt[c][:, tt * 128:(tt + 1) * 128]
                            nc.tensor.matmul(ps_k, lhsT, wk[:, c, :],
                                             start=(c == 0), stop=(c == 3))
                            nc.tensor.matmul(ps_v, lhsT, wv[:, c, :],
                                             start=(c == 0), stop=(c == 3))
                        k_sb = p1k.tile([128, 512], F16, tag="k")
                        nc.scalar.activation(out=k_sb, in_=ps_k, func=AF.Sigmoid)
                        if debug_taps and gt == 0:
                            nc.gpsimd.dma_start(out=dbg_k[:, :], in_=k_sb)
                        v_aug = p1v.tile([128, 4, 129], F16, tag="vaug")
                        mcol = mask_sb[:, gt:gt + 1]
                        for p in range(4):
                            nc.vector.tensor_scalar_mul(
                                out=v_aug[:, p, 0:128],
                                in0=ps_v[:, p * 128:(p + 1) * 128],
                                scalar1=mcol)
                        nc.gpsimd.memset(v_aug[:, :, 128:129], 1.0)
                        for p in range(4):
                            nc.tensor.matmul(
                                kvacc[p][:, :],
                                k_sb[:, p * 128:(p + 1) * 128],
                                v_aug[:, p, :],
                                start=(gt == 0), stop=(gt == n_tiles - 1))

                # ---- pairwise AllReduce of kv/ksum ----
                cc_sb = singles.tile([128, 516], F32, tag="ccsb")
                for q in range(4):
                    nc.vector.tensor_copy(out=cc_sb[:, q * 129:(q + 1) * 129],
                                          in_=kvacc[q][:, :])
                cc_in = dram.tile([128, 516], F32, tag="ccin")
                cc_out = dram.tile([128, 516], F32, tag="ccout")
                nc.sync.dma_start(out=cc_in, in_=cc_sb)
                nc.gpsimd.collective_compute(
                    "AllReduce", mybir.AluOpType.add,
                    replica_groups=[[0, 1], [2, 3], [4, 5], [6, 7]],
                    ins=[cc_in.opt()], outs=[cc_out.opt()])
                cc_rd = singles.tile([128, 516], F32, tag="ccrd")
                nc.sync.dma_start(out=cc_rd, in_=cc_out)
                if debug_taps:
                    nc.sync.dma_start(out=dbg_cc[:, :], in_=cc_rd)

            # block-diagonal rhs tiles for num/den:
            # cols: [kv_h0 (64) | ks_h0 | kv_h1 (64) | ks_h1], rows = d of
            # h0 (0:64) then d of h1 (64:128); zero on the off-blocks.
            numrhs = []
            for pp in range(4):
                t = singles.tile([128, 130], F16, tag=f"numrhs{pp}")
                nc.vector.memset(t, 0.0)
                base = pp * 129
                nc.vector.tensor_copy(out=t[0:64, 0:64],
                                      in_=cc_rd[0:64, base:base + 64])
                nc.vector.tensor_copy(out=t[0:64, 64:65],
                                      in_=cc_rd[0:64, base + 128:base + 129])
                nc.vector.tensor_copy(out=t[64:128, 65:129],
                                      in_=cc_rd[64:128, base + 64:base + 128])
                nc.vector.tensor_copy(out=t[64:128, 129:130],
                                      in_=cc_rd[64:128, base + 128:base + 129])
                numrhs.append(t)

            # ============================ PASS 2 ============================
            with (
                tc.tile_pool(name="p2xt", bufs=2) as p2xt,
                tc.tile_pool(name="p2kt", bufs=2) as p2kt,
                tc.tile_pool(name="p2h", bufs=6) as p2h,
                tc.tile_pool(name="p2attn", bufs=6) as p2attn,
                tc.tile_pool(name="p2ln", bufs=3) as p2ln,
                tc.tile_pool(name="p2pre", bufs=3) as p2pre,
                tc.tile_pool(name="p2preT", bufs=3) as p2preT,
                tc.tile_pool(name="p2out", bufs=3) as p2out,
                tc.tile_pool(name="p2sm", bufs=3) as p2sm,
                tc.tile_pool(name="pskt", bufs=2, space="PSUM") as pskt,
                tc.tile_pool(name="psh", bufs=2, space="PSUM") as psh,
                tc.tile_pool(name="psnum", bufs=1, space="PSUM") as psnum,
                tc.tile_pool(name="pstr", bufs=1, space="PSUM") as pstr,
                tc.tile_pool(name="psf", bufs=1, space="PSUM") as psf,
            ):
                for blk in range(n_blocks):
                    xt = []
                    for c in range(4):
                        t = p2xt.tile([128, 512], F16, tag=f"xt{c}")
                        nc.sync.dma_start(
                            out=t, in_=xT[c * 128:(c + 1) * 128,
                                          blk * 512:(blk + 1) * 512])
                        xt.append(t)
                    # kT (dh-major; chunk pp = heads 2pp, 2pp+1)
                    kt = []
                    for pp in range(4):
                        ps_kt = pskt.tile([128, 512], F32, tag="pskt")
                        for c in range(4):
                            nc.tensor.matmul(ps_kt,
                                             wk[:, c, pp * 128:(pp + 1) * 128],
                                             xt[c], start=(c == 0), stop=(c == 3))
                        t = p2kt.tile([128, 512], F16, tag=f"kt{pp}")
                        nc.scalar.activation(out=t, in_=ps_kt, func=AF.Sigmoid)
                        if debug_taps and blk == 0 and pp == 0:
                            nc.gpsimd.dma_start(out=dbg_kt[:, :], in_=t)
                        kt.append(t)

                    mvbuf = p2sm.tile([128, 4, 2], F32, tag="mv")
                    attn_t = []
                    h_t = []
                    for tt in range(4):
                        # h = x @ Wh + (bh + beta)
                        ps_h = psh.tile([128, 512], F32, tag="psh")
                        for c in range(4):
                            nc.tensor.matmul(ps_h,
                                             xt[c][:, tt * 128:(tt + 1) * 128],
                                             wh[:, c, :],
                                             start=(c == 0), stop=(c == 3))
                        h_sb = p2h.tile([128, 512], F32, tag="h")
                        nc.vector.tensor_add(out=h_sb, in0=ps_h, in1=bhb_r)
                        h_t.append(h_sb)
                        # num/den: pn0 holds pairs 0,1; pn1 pairs 2,3
                        pn0 = psnum.tile([128, 2, 130], F32, tag="pn0")
                        pn1 = psnum.tile([128, 2, 130], F32, tag="pn1")
                        for pp in range(4):
                            pn = pn0 if pp < 2 else pn1
                            nc.tensor.matmul(
                                pn[:, pp % 2, :],
                                kt[pp][:, tt * 128:(tt + 1) * 128],
                                numrhs[pp], start=True, stop=True)
                        dr = p2sm.tile([128, 8], F32, tag="dr")
                        nc.vector.reciprocal(out=dr[:, 0:4],
                                             in_=_strided_cols(pn0, 64, 65, 4))
                        nc.vector.reciprocal(out=dr[:, 4:8],
                                             in_=_strided_cols(pn1, 64, 65, 4))
                        attn = p2attn.tile([128, 512], F32, tag="attn")
                        for h8 in range(8):
                            pn = pn0 if h8 < 4 else pn1
                            r = (h8 // 2) % 2
                            e = h8 % 2
                            nc.vector.tensor_scalar_mul(
                                out=attn[:, h8 * 64:(h8 + 1) * 64],
                                in0=pn[:, r, e * 65:e * 65 + 64],
                                scalar1=dr[:, h8:h8 + 1])
                        st = p2sm.tile([128, 6], F32, tag="st")
                        nc.vector.bn_stats(out=st, in_=attn)
                        nc.vector.bn_aggr(out=mvbuf[:, tt, :], in_=st)
                        if debug_taps and blk == 0 and tt == 0:
                            nc.sync.dma_start(out=dbg_attn[:, :], in_=attn)
                            nc.sync.dma_start(out=dbg_h[:, :], in_=h_sb)
                        attn_t.append(attn)

                    # batched LN scale factors for the 4 tiles of this block
                    stdb = p2sm.tile([128, 4], F32, tag="stdb")
                    rstdb = p2sm.tile([128, 4], F32, tag="rstdb")
                    nmb = p2sm.tile([128, 4], F32, tag="nmb")
                    nc.scalar.activation(out=stdb,
                                         in_=_strided_cols(mvbuf, 1, 2, 4),
                                         func=AF.Sqrt, bias=eps_t, scale=1.0)
                    nc.vector.reciprocal(out=rstdb, in_=stdb)
                    nc.vector.tensor_mul(out=nmb,
                                         in0=_strided_cols(mvbuf, 0, 2, 4),
                                         in1=rstdb)
                    nc.vector.tensor_scalar_mul(out=nmb, in0=nmb, scalar1=-1.0)

                    for tt in range(4):
                        ln_sb = p2ln.tile([128, 512], F32, tag="ln")
                        nc.scalar.activation(out=ln_sb, in_=attn_t[tt],
                                             func=AF.Identity,
                                             bias=nmb[:, tt:tt + 1],
                                             scale=rstdb[:, tt:tt + 1])
                        if apply_ln_affine:
                            nc.vector.tensor_mul(out=ln_sb, in0=ln_sb, in1=lng_r)
                            nc.vector.tensor_add(out=ln_sb, in0=ln_sb, in1=lnb_r)
                        pre = p2pre.tile([128, 512], F16, tag="pre")
                        nc.vector.tensor_mul(out=pre, in0=ln_sb, in1=h_t[tt])
                        if debug_taps and blk == 0 and tt == 0:
                            nc.gpsimd.dma_start(out=dbg_pre[:, :], in_=pre)
                        # transpose pre -> [ch, n] and matmul with Wo
                        tp = pstr.tile([128, 512], F16, tag="tp")
                        for c in range(4):
                            nc.tensor.transpose(tp[:, c * 128:(c + 1) * 128],
                                                pre[:, c * 128:(c + 1) * 128],
                                                ident)
                        preT = p2preT.tile([128, 4, 128], F16, tag="preT")
                        nc.vector.tensor_copy(out=preT.rearrange("p a b -> p (a b)"),
                                              in_=tp[:, :])
                        ps_f = psf.tile([128, 512], F32, tag="psf")
                        nc.tensor.matmul(ps_f, ones_lhsT, bo_sb,
                                         start=True, stop=False)
                        for c in range(4):
                            nc.tensor.matmul(ps_f, preT[:, c, :], wo[:, c, :],
                                             start=False, stop=(c == 3))
                        osb = p2out.tile([128, 512], F32, tag="osb")
                        nc.vector.tensor_relu(out=osb, in_=ps_f)
                        r0 = blk * 512 + tt * 128
                        nc.sync.dma_start(out=out_d[r0:r0 + 128, :], in_=osb)
    nc.finalize()
    return nc


_PROGRAM_CACHE = {}


def _get_program(rows, apply_ln_affine):
    key = (rows, apply_ln_affine)
    if key not in _PROGRAM_CACHE:
        _PROGRAM_CACHE[key] = build_program(rows, apply_ln_affine)
    return _PROGRAM_CACHE[key]


def run_sharded(x, mask, Wh, bh, Wk, Wv, ln_g, ln_b, Wo, bo, trace=False):
    """Shard on host, run the SPMD program, gather. x may have any N that is
    a multiple of 1024 (rows/core multiple of 512)."""
    from concourse.bass_utils import run_bass_kernel_spmd

    x = np.asarray(x, dtype=np.float32)
    mask = np.asarray(mask)
    b, n, _ = x.shape
    assert b == B and n % (2 * 512) == 0
    rows = n // 2

    P = _perm()
    lng_p = np.ascontiguousarray(np.asarray(ln_g, np.float32)[P])
    lnb_p = np.ascontiguousarray(np.asarray(ln_b, np.float32)[P])
    apply_ln = not (np.all(lng_p == 1.0) and np.all(lnb_p == 0.0))

    wk_p = np.ascontiguousarray(np.asarray(Wk, np.float32)[:, P]).astype(np.float16)
    wv_p = np.ascontiguousarray(np.asarray(Wv, np.float32)[:, P]).astype(np.float16)
    wh_p = np.ascontiguousarray(np.asarray(Wh, np.float32)[:, P]).astype(np.float16)
    wo_p = np.ascontiguousarray(np.asarray(Wo, np.float32)[P, :]).astype(np.float16)
    bhb = np.ascontiguousarray(
        (np.asarray(bh, np.float32)[P] + BETA).reshape(1, INNER))
    bo_r = np.asarray(bo, np.float32).reshape(1, INNER).astype(np.float16)

    in_maps = []
    for c in range(N_CORES):
        bb, ss = c // 2, c % 2
        xs = x[bb, ss * rows:(ss + 1) * rows, :]
        xTs = np.ascontiguousarray(xs.T).astype(np.float16)
        ms = np.asarray(mask[bb, ss * rows:(ss + 1) * rows], np.float32)
        mtiles = np.ascontiguousarray(ms.reshape(rows // 128, 128).T)
        im = {"xT": xTs, "maskf": mtiles, "wk": wk_p, "wv": wv_p,
              "wh": wh_p, "wo": wo_p, "bhb": bhb, "bo_r": bo_r}
        if apply_ln:
            im["lng"] = lng_p.reshape(1, INNER)
            im["lnb"] = lnb_p.reshape(1, INNER)
        in_maps.append(im)

    nc = _get_program(rows, apply_ln)
    res = run_bass_kernel_spmd(nc, in_maps, list(range(N_CORES)), trace=trace)

    out = np.empty((B, n, INNER), np.float32)
    for c in range(N_CORES):
        bb, ss = c // 2, c % 2
        out[bb, ss * rows:(ss + 1) * rows, :] = res.results[c]["out"]
    return out


def kernel(x, mask, Wh, bh, Wk, Wv, ln_g, ln_b, Wo, bo):
    return run_sharded(x, mask, Wh, bh, Wk, Wv, ln_g, ln_b, Wo, bo)


# revision 23
# speedup vs baseline: 28911.9382x; 28911.9382x over previous
"""PolynormerAttention Trainium2 kernel (8 NeuronCores).

Contract: kernel(**inputs) takes the FULL tensors (x [4,16384,512] f32,
mask [4,16384] bool, weights) and returns the FULL output [4,16384,512] f32.

Sharding: core c handles batch c//2, row-half c%2 (8192 rows). The per-head
reductions kv [64,64,H] / k_sum [64,H] are completed with an in-kernel
pairwise AllReduce (replica groups [[0,1],[2,3],[4,5],[6,7]]).

Host-side prep (free — only HW kernel time matters):
  - head-major channel permutation (flat' = h*64+d) applied to Wk/Wv/Wh
    columns, Wo rows, bh/ln_g/ln_b, so each head is 64 contiguous columns.
  - x is pre-transposed per core (xT [512, 8192]) so the kernel never
    transposes x on-chip (contraction over CH needs CH on partitions).
  - matmul operands are fp16 (fp32 PSUM accumulation; all values far inside
    fp16 range); elementwise math stays fp32.

Device program (identical on all cores):
  pass 1: k = sigmoid(x@Wk), v = (x@Wv)*mask as [n, dh] tiles;
          kv/ksum accumulated in PSUM over all rows via per-head-pair
          matmuls with rhs [v_pair | ones]; pairwise AllReduce.
          (each of the 4 pair accumulators gets its OWN psum bank — two
          open accumulation groups must never share a bank.)
  pass 2: kT = sigmoid((x@Wk).T) computed directly dh-major (lhsT = Wk);
          h = x@Wh + (bh+beta); num/den in one matmul per head-pair with a
          block-diagonal [kv|ksum] rhs; attn = num * recip(den); layernorm
          (Sqrt batched per 512-row block to limit ACT table switches);
          * h; PE-transpose; @ Wo (+bo folded in as a K=1 matmul); relu.
"""

import numpy as np

HEADS = 8
HEAD_CH = 64
CH = 512
INNER = 512
BETA = 0.9
B = 4
N = 16384
N_CORES = 8
LN_EPS = 1e-5


def _perm():
    # new flat index j = h*64 + d  <-  old flat index d*8 + h
    j = np.arange(INNER)
    h = j // HEAD_CH
    d = j % HEAD_CH
    return d * HEADS + h


def _strided_cols(t, start, step, count):
    """AP selecting flattened-free columns start, start+step, ... of a tile."""
    import concourse.bass as bass
    a = t[:, 0:1]
    p_ent = a.ap[0]
    f_stride = a.ap[-1][0]
    return bass.AP(tensor=a.tensor, offset=a.offset + start * f_stride,
                   ap=[p_ent, [step * f_stride, count]])


def build_program(rows, apply_ln_affine, debug_taps=False, reps=1):
    """Build the per-core Bass program for `rows` rows/core.

    reps > 1 repeats the whole (idempotent) computation for benchmarking:
    HW-time-per-rep = (wall(reps=a) - wall(reps=b)) / (a - b).
    """
    import concourse.bass as bass
    import concourse.tile as tile
    from concourse import mybir, bacc
    from concourse.masks import make_identity

    F32 = mybir.dt.float32
    F16 = mybir.dt.float16
    AF = mybir.ActivationFunctionType

    assert rows % 512 == 0
    n_blocks = rows // 512

    nc = bacc.Bacc("TRN2", target_bir_lowering=False, debug=False,
                   num_devices=N_CORES)

    xT = nc.dram_tensor("xT", [CH, rows], F16, kind="ExternalInput")
    maskf = nc.dram_tensor("maskf", [128, rows // 128], F32, kind="ExternalInput")
    wk_d = nc.dram_tensor("wk", [CH, INNER], F16, kind="ExternalInput")
    wv_d = nc.dram_tensor("wv", [CH, INNER], F16, kind="ExternalInput")
    wh_d = nc.dram_tensor("wh", [CH, INNER], F16, kind="ExternalInput")
    wo_d = nc.dram_tensor("wo", [INNER, INNER], F16, kind="ExternalInput")
    bhb_d = nc.dram_tensor("bhb", [1, INNER], F32, kind="ExternalInput")
    bo_d = nc.dram_tensor("bo_r", [1, INNER], F16, kind="ExternalInput")
    if apply_ln_affine:
        lng_d = nc.dram_tensor("lng", [1, INNER], F32, kind="ExternalInput")
        lnb_d = nc.dram_tensor("lnb", [1, INNER], F32, kind="ExternalInput")
    out_d = nc.dram_tensor("out", [rows, INNER], F32, kind="ExternalOutput")
    if debug_taps:
        dbg_cc = nc.dram_tensor("dbg_cc", [128, 516], F32, kind="ExternalOutput")
        dbg_attn = nc.dram_tensor("dbg_attn", [128, INNER], F32,
                                  kind="ExternalOutput")
        dbg_h = nc.dram_tensor("dbg_h", [128, INNER], F32, kind="ExternalOutput")
        dbg_k = nc.dram_tensor("dbg_k", [128, INNER], F32, kind="ExternalOutput")
        dbg_kt = nc.dram_tensor("dbg_kt", [128, INNER], F32, kind="ExternalOutput")
        dbg_pre = nc.dram_tensor("dbg_pre", [128, INNER], F32,
                                 kind="ExternalOutput")

    def bcast128(ap_1xN):
        return bass.AP(tensor=ap_1xN.tensor, offset=ap_1xN.offset,
                       ap=[[0, 128]] + list(ap_1xN.ap[1:]))

    with tile.TileContext(nc) as tc:
        with (
            tc.tile_pool(name="singles", bufs=1) as singles,
            tc.tile_pool(name="dram", bufs=1, space="DRAM") as dram,
        ):
            # ---- weights / constants ----
            wk = singles.tile([128, 4, INNER], F16, tag="wk")
            wv = singles.tile([128, 4, INNER], F16, tag="wv")
            wh = singles.tile([128, 4, INNER], F16, tag="wh")
            wo = singles.tile([128, 4, INNER], F16, tag="wo")
            for w_sb, w_dram in ((wk, wk_d), (wv, wv_d), (wh, wh_d), (wo, wo_d)):
                nc.sync.dma_start(
                    out=w_sb,
                    in_=w_dram[:, :].rearrange("(c p) i -> p c i", p=128))
            mask_sb = singles.tile([128, rows // 128], F32, tag="mask")
            nc.sync.dma_start(out=mask_sb, in_=maskf[:, :])
            bo_sb = singles.tile([1, INNER], F16, tag="bo")
            nc.sync.dma_start(out=bo_sb, in_=bo_d[:, :])
            bhb_r = singles.tile([128, INNER], F32, tag="bhbr")
            nc.gpsimd.dma_start(out=bhb_r, in_=bcast128(bhb_d[:, :]))
            if apply_ln_affine:
                lng_r = singles.tile([128, INNER], F32, tag="lngr")
                lnb_r = singles.tile([128, INNER], F32, tag="lnbr")
                nc.gpsimd.dma_start(out=lng_r, in_=bcast128(lng_d[:, :]))
                nc.gpsimd.dma_start(out=lnb_r, in_=bcast128(lnb_d[:, :]))
            ident = singles.tile([128, 128], F16, tag="ident")
            make_identity(nc, ident)
            ones_lhsT = singles.tile([1, 128], F16, tag="ones1")
            nc.gpsimd.memset(ones_lhsT, 1.0)
            eps_t = singles.tile([128, 1], F32, tag="eps")
            nc.vector.memset(eps_t, LN_EPS)

            for rep in range(reps):
                emit_iteration(
                    nc, tc, bass, tile, mybir, rep, rows, n_blocks,
                    apply_ln_affine, debug_taps and rep == 0, singles, dram,
                    xT, out_d, wk, wv, wh, wo, mask_sb, bo_sb, bhb_r,
                    lng_r if apply_ln_affine else None,
                    lnb_r if apply_ln_affine else None,
                    ident, ones_lhsT, eps_t,
                    locals().get("dbg_cc"), locals().get("dbg_attn"),
                    locals().get("dbg_h"), locals().get("dbg_k"),
                    locals().get("dbg_kt"), locals().get("dbg_pre"))
    nc.finalize()
    return nc


def emit_iteration(nc, tc, bass, tile, mybir, rep, rows, n_blocks,
                   apply_ln_affine, debug_taps, singles, dram,
                   xT, out_d, wk, wv, wh, wo, mask_sb, bo_sb, bhb_r,
                   lng_r, lnb_r, ident, ones_lhsT, eps_t,
                   dbg_cc, dbg_attn, dbg_h, dbg_k, dbg_kt, dbg_pre):
    F32 = mybir.dt.float32
    F16 = mybir.dt.float16
    AF = mybir.ActivationFunctionType
    R = f"r{rep}"

    # ============================ PASS 1 ============================
    with (
        tc.tile_pool(name=f"p1xt{R}", bufs=2) as p1xt,
        tc.tile_pool(name=f"p1k{R}", bufs=3) as p1k,
        tc.tile_pool(name=f"p1v{R}", bufs=3) as p1v,
        tc.tile_pool(name=f"ps1{R}", bufs=2, space="PSUM") as ps1,
        tc.tile_pool(name=f"pskv{R}", bufs=1, space="PSUM") as pskv,
    ):
        kvacc = [pskv.tile([128, 129], F32, tag=f"kvacc{q}",
                           name=f"kvacc{q}{R}")
                 for q in range(4)]
        n_tiles = 4 * n_blocks
        for blk in range(n_blocks):
            xt = []
            for c in range(4):
                t = p1xt.tile([128, 512], F16, tag=f"xt{c}", name=f"p1x{c}{R}")
                nc.sync.dma_start(
                    out=t, in_=xT[c * 128:(c + 1) * 128,
                                  blk * 512:(blk + 1) * 512])
                xt.append(t)
            for tt in range(4):
                gt = blk * 4 + tt
                ps_k = ps1.tile([128, 512], F32, tag="psk", name=f"psk{R}")
                ps_v = ps1.tile([128, 512], F32, tag="psv", name=f"psv{R}")
                for c in range(4):
                    lhsT = xt[c][:, tt * 128:(tt + 1) * 128]
                    nc.tensor.matmul(ps_k, lhsT, wk[:, c, :],
                                     start=(c == 0), stop=(c == 3))
                    nc.tensor.matmul(ps_v, lhsT, wv[:, c, :],
                                     start=(c == 0), stop=(c == 3))
                k_sb = p1k.tile([128, 512], F16, tag="k", name=f"k_sb{R}")
                nc.scalar.activation(out=k_sb, in_=ps_k, func=AF.Sigmoid)
                if debug_taps and gt == 0:
                    nc.gpsimd.dma_start(out=dbg_k[:, :], in_=k_sb)
                v_aug = p1v.tile([128, 4, 129], F16, tag="vaug",
                                 name=f"v_aug{R}")
                mcol = mask_sb[:, gt:gt + 1]
                for p in range(4):
                    nc.vector.tensor_scalar_mul(
                        out=v_aug[:, p, 0:128],
                        in0=ps_v[:, p * 128:(p + 1) * 128],
                        scalar1=mcol)
                nc.gpsimd.memset(v_aug[:, :, 128:129], 1.0)
                for p in range(4):
                    nc.tensor.matmul(
                        kvacc[p][:, :],
                        k_sb[:, p * 128:(p + 1) * 128],
                        v_aug[:, p, :],
                        start=(gt == 0), stop=(gt == n_tiles - 1))

        # ---- pairwise AllReduce of kv/ksum ----
        cc_sb = singles.tile([128, 516], F32, tag="ccsb", name=f"cc_sb{R}")
        for q in range(4):
            nc.vector.tensor_copy(out=cc_sb[:, q * 129:(q + 1) * 129],
                                  in_=kvacc[q][:, :])
        cc_in = dram.tile([128, 516], F32, tag="ccin", name=f"cc_in{R}")
        cc_out = dram.tile([128, 516], F32, tag="ccout", name=f"cc_out{R}")
        nc.sync.dma_start(out=cc_in, in_=cc_sb)
        nc.gpsimd.collective_compute(
            "AllReduce", mybir.AluOpType.add,
            replica_groups=[[0, 1], [2, 3], [4, 5], [6, 7]],
            ins=[cc_in.opt()], outs=[cc_out.opt()])
        cc_rd = singles.tile([128, 516], F32, tag="ccrd", name=f"cc_rd{R}")
        nc.sync.dma_start(out=cc_rd, in_=cc_out)
        if debug_taps:
            nc.sync.dma_start(out=dbg_cc[:, :], in_=cc_rd)

    # block-diagonal rhs tiles for num/den:
    # cols [kv_h0 | ks_h0 | kv_h1 | ks_h1]; rows = d of h0 (0:64) then
    # d of h1 (64:128); zeros off-block.
    numrhs = []
    for pp in range(4):
        t = singles.tile([128, 130], F16, tag=f"numrhs{pp}",
                         name=f"numrhs{pp}{R}")
        nc.vector.memset(t, 0.0)
        base = pp * 129
        nc.vector.tensor_copy(out=t[0:64, 0:64],
                              in_=cc_rd[0:64, base:base + 64])
        nc.vector.tensor_copy(out=t[0:64, 64:65],
                              in_=cc_rd[0:64, base + 128:base + 129])
        nc.vector.tensor_copy(out=t[64:128, 65:129],
                              in_=cc_rd[64:128, base + 64:base + 128])
        nc.vector.tensor_copy(out=t[64:128, 129:130],
                              in_=cc_rd[64:128, base + 128:base + 129])
        numrhs.append(t)

    # ============================ PASS 2 ============================
    with (
        tc.tile_pool(name=f"p2xt{R}", bufs=2) as p2xt,
        tc.tile_pool(name=f"p2kt{R}", bufs=2) as p2kt,
        tc.tile_pool(name=f"p2h{R}", bufs=6) as p2h,
        tc.tile_pool(name=f"p2attn{R}", bufs=6) as p2attn,
        tc.tile_pool(name=f"p2ln{R}", bufs=3) as p2ln,
        tc.tile_pool(name=f"p2pre{R}", bufs=3) as p2pre,
        tc.tile_pool(name=f"p2preT{R}", bufs=3) as p2preT,
        tc.tile_pool(name=f"p2out{R}", bufs=3) as p2out,
        tc.tile_pool(name=f"p2sm{R}", bufs=3) as p2sm,
        tc.tile_pool(name=f"pskt{R}", bufs=2, space="PSUM") as pskt,
        tc.tile_pool(name=f"psh{R}", bufs=2, space="PSUM") as psh,
        tc.tile_pool(name=f"psnum{R}", bufs=1, space="PSUM") as psnum,
        tc.tile_pool(name=f"pstr{R}", bufs=1, space="PSUM") as pstr,
        tc.tile_pool(name=f"psf{R}", bufs=1, space="PSUM") as psf,
    ):
        for blk in range(n_blocks):
            xt = []
            for c in range(4):
                t = p2xt.tile([128, 512], F16, tag=f"xt{c}", name=f"p2x{c}{R}")
                nc.sync.dma_start(
                    out=t, in_=xT[c * 128:(c + 1) * 128,
                                  blk * 512:(blk + 1) * 512])
                xt.append(t)
            # kT (dh-major; chunk pp = heads 2pp, 2pp+1)
            kt = []
            for pp in range(4):
                ps_kt = pskt.tile([128, 512], F32, tag="pskt",
                                  name=f"ps_kt{R}")
                for c in range(4):
                    nc.tensor.matmul(ps_kt,
                                     wk[:, c, pp * 128:(pp + 1) * 128],
                                     xt[c], start=(c == 0), stop=(c == 3))
                t = p2kt.tile([128, 512], F16, tag=f"kt{pp}",
                              name=f"kt{pp}{R}")
                nc.scalar.activation(out=t, in_=ps_kt, func=AF.Sigmoid)
                if debug_taps and blk == 0 and pp == 0:
                    nc.gpsimd.dma_start(out=dbg_kt[:, :], in_=t)
                kt.append(t)

            mvbuf = p2sm.tile([128, 4, 2], F32, tag="mv", name=f"mvbuf{R}")
            attn_t = []
            h_t = []
            for tt in range(4):
                # h = x @ Wh + (bh + beta)
                ps_h = psh.tile([128, 512], F32, tag="psh", name=f"ps_h{R}")
                for c in range(4):
                    nc.tensor.matmul(ps_h,
                                     xt[c][:, tt * 128:(tt + 1) * 128],
                                     wh[:, c, :],
                                     start=(c == 0), stop=(c == 3))
                h_sb = p2h.tile([128, 512], F32, tag="h", name=f"h_sb{R}")
                nc.vector.tensor_add(out=h_sb, in0=ps_h, in1=bhb_r)
                h_t.append(h_sb)
                # num/den: pn0 holds pairs 0,1; pn1 pairs 2,3
                pn0 = psnum.tile([128, 2, 130], F32, tag="pn0",
                                 name=f"pn0{R}")
                pn1 = psnum.tile([128, 2, 130], F32, tag="pn1",
                                 name=f"pn1{R}")
                for pp in range(4):
                    pn = pn0 if pp < 2 else pn1
                    nc.tensor.matmul(
                        pn[:, pp % 2, :],
                        kt[pp][:, tt * 128:(tt + 1) * 128],
                        numrhs[pp], start=True, stop=True)
                dr = p2sm.tile([128, 8], F32, tag="dr", name=f"dr{R}")
                nc.vector.reciprocal(out=dr[:, 0:4],
                                     in_=_strided_cols(pn0, 64, 65, 4))
                nc.vector.reciprocal(out=dr[:, 4:8],
                                     in_=_strided_cols(pn1, 64, 65, 4))
                attn = p2attn.tile([128, 512], F32, tag="attn",
                                   name=f"attn{R}")
                for h8 in range(8):
                    pn = pn0 if h8 < 4 else pn1
                    r = (h8 // 2) % 2
                    e = h8 % 2
                    nc.vector.tensor_scalar_mul(
                        out=attn[:, h8 * 64:(h8 + 1) * 64],
                        in0=pn[:, r, e * 65:e * 65 + 64],
                        scalar1=dr[:, h8:h8 + 1])
                st = p2sm.tile([128, 6], F32, tag="st", name=f"st{R}")
                nc.vector.bn_stats(out=st, in_=attn)
                nc.vector.bn_aggr(out=mvbuf[:, tt, :], in_=st)
                if debug_taps and blk == 0 and tt == 0:
                    nc.sync.dma_start(out=dbg_attn[:, :], in_=attn)
                    nc.sync.dma_start(out=dbg_h[:, :], in_=h_sb)
                attn_t.append(attn)

            # batched LN scale factors for the block's 4 tiles
            stdb = p2sm.tile([128, 4], F32, tag="stdb", name=f"stdb{R}")
            rstdb = p2sm.tile([128, 4], F32, tag="rstdb", name=f"rstdb{R}")
            nmb = p2sm.tile([128, 4], F32, tag="nmb", name=f"nmb{R}")
            nc.scalar.activation(out=stdb,
                                 in_=_strided_cols(mvbuf, 1, 2, 4),
                                 func=AF.Sqrt, bias=eps_t, scale=1.0)
            nc.vector.reciprocal(out=rstdb, in_=stdb)
            nc.vector.tensor_mul(out=nmb,
                                 in0=_strided_cols(mvbuf, 0, 2, 4),
                                 in1=rstdb)
            nc.vector.tensor_scalar_mul(out=nmb, in0=nmb, scalar1=-1.0)

            for tt in range(4):
                ln_sb = p2ln.tile([128, 512], F32, tag="ln", name=f"ln_sb{R}")
                nc.scalar.activation(out=ln_sb, in_=attn_t[tt],
                                     func=AF.Identity,
                                     bias=nmb[:, tt:tt + 1],
                                     scale=rstdb[:, tt:tt + 1])
                if apply_ln_affine:
                    nc.vector.tensor_mul(out=ln_sb, in0=ln_sb, in1=lng_r)
                    nc.vector.tensor_add(out=ln_sb, in0=ln_sb, in1=lnb_r)
                pre = p2pre.tile([128, 512], F16, tag="pre", name=f"pre{R}")
                nc.vector.tensor_mul(out=pre, in0=ln_sb, in1=h_t[tt])
                if debug_taps and blk == 0 and tt == 0:
                    nc.gpsimd.dma_start(out=dbg_pre[:, :], in_=pre)
                # transpose pre -> [ch, n] and matmul with Wo
                tp = pstr.tile([128, 512], F16, tag="tp", name=f"tp{R}")
                for c in range(4):
                    nc.tensor.transpose(tp[:, c * 128:(c + 1) * 128],
                                        pre[:, c * 128:(c + 1) * 128],
                                        ident)
                preT = p2preT.tile([128, 4, 128], F16, tag="preT",
                                   name=f"preT{R}")
                nc.vector.tensor_copy(out=preT.rearrange("p a b -> p (a b)"),
                                      in_=tp[:, :])
                ps_f = psf.tile([128, 512], F32, tag="psf", name=f"ps_f{R}")
                nc.tensor.matmul(ps_f, ones_lhsT, bo_sb,
                                 start=True, stop=False)
                for c in range(4):
                    nc.tensor.matmul(ps_f, preT[:, c, :], wo[:, c, :],
                                     start=False, stop=(c == 3))
                osb = p2out.tile([128, 512], F32, tag="osb", name=f"osb{R}")
                nc.vector.tensor_relu(out=osb, in_=ps_f)
                r0 = blk * 512 + tt * 128
                nc.sync.dma_start(out=out_d[r0:r0 + 128, :], in_=osb)


_PROGRAM_CACHE = {}


def _get_program(rows, apply_ln_affine, reps=1):
    key = (rows, apply_ln_affine, reps)
    if key not in _PROGRAM_CACHE:
        _PROGRAM_CACHE[key] = build_program(rows, apply_ln_affine, reps=reps)
    return _PROGRAM_CACHE[key]


def prep_inputs(x, mask, Wh, bh, Wk, Wv, ln_g, ln_b, Wo, bo):
    """Host-side sharding/permutation. Returns (in_maps, rows, apply_ln)."""
    x = np.asarray(x, dtype=np.float32)
    mask = np.asarray(mask)
    b, n, _ = x.shape
    assert b == B and n % (2 * 512) == 0
    rows = n // 2

    P = _perm()
    lng_p = np.ascontiguousarray(np.asarray(ln_g, np.float32)[P])
    lnb_p = np.ascontiguousarray(np.asarray(ln_b, np.float32)[P])
    apply_ln = not (np.all(lng_p == 1.0) and np.all(lnb_p == 0.0))

    wk_p = np.ascontiguousarray(np.asarray(Wk, np.float32)[:, P]).astype(np.float16)
    wv_p = np.ascontiguousarray(np.asarray(Wv, np.float32)[:, P]).astype(np.float16)
    wh_p = np.ascontiguousarray(np.asarray(Wh, np.float32)[:, P]).astype(np.float16)
    wo_p = np.ascontiguousarray(np.asarray(Wo, np.float32)[P, :]).astype(np.float16)
    bhb = np.ascontiguousarray(
        (np.asarray(bh, np.float32)[P] + BETA).reshape(1, INNER))
    bo_r = np.asarray(bo, np.float32).reshape(1, INNER).astype(np.float16)

    in_maps = []
    for c in range(N_CORES):
        bb, ss = c // 2, c % 2
        xs = x[bb, ss * rows:(ss + 1) * rows, :]
        xTs = np.ascontiguousarray(xs.T).astype(np.float16)
        ms = np.asarray(mask[bb, ss * rows:(ss + 1) * rows], np.float32)
        mtiles = np.ascontiguousarray(ms.reshape(rows // 128, 128).T)
        im = {"xT": xTs, "maskf": mtiles, "wk": wk_p, "wv": wv_p,
              "wh": wh_p, "wo": wo_p, "bhb": bhb, "bo_r": bo_r}
        if apply_ln:
            im["lng"] = lng_p.reshape(1, INNER)
            im["lnb"] = lnb_p.reshape(1, INNER)
        in_maps.append(im)
    return in_maps, rows, apply_ln


def run_sharded(x, mask, Wh, bh, Wk, Wv, ln_g, ln_b, Wo, bo):
    from concourse.bass_utils import run_bass_kernel_spmd

    in_maps, rows, apply_ln = prep_inputs(x, mask, Wh, bh, Wk, Wv,
                                          ln_g, ln_b, Wo, bo)
    nc = _get_program(rows, apply_ln)
    res = run_bass_kernel_spmd(nc, in_maps, list(range(N_CORES)))

    n = 2 * rows
    out = np.empty((B, n, INNER), np.float32)
    for c in range(N_CORES):
        bb, ss = c // 2, c % 2
        out[bb, ss * rows:(ss + 1) * rows, :] = res.results[c]["out"]
    return out


def kernel(x, mask, Wh, bh, Wk, Wv, ln_g, ln_b, Wo, bo):
    return run_sharded(x, mask, Wh, bh, Wk, Wv, ln_g, ln_b, Wo, bo)
